# revision 4
# baseline (speedup 1.0000x reference)
"""Trainium2 Bass kernel for a top-2-of-4 MoE layer (DSMoE).

Contract: kernel(x, gate_w, w1, w2) -> (out, router_sparse) with
  x            [8, 2048, 512] f32
  gate_w       [512, 4]       f32
  w1           [4, 512, 2048] f32
  w2           [4, 2048, 512] f32
  out          [8, 2048, 512] f32
  router_sparse[16384, 4]     f32

Sharding: data-parallel over tokens, 2048 tokens per core on 8 cores,
weights replicated (no collectives). Each core runs every expert on its
token shard and combines with the sparse router weights — mathematically
identical to top-2 dispatch (the reference itself is formulated densely).

On-chip layout: activations are kept feature-major ([C, T] / [H, T]) so
both expert matmuls consume weights in their natural layout with zero
transposes; the second matmul flips back to token-major by using the
hidden activations as the stationary operand. Expert matmuls run in
float32r (full PE rate, ~1.5e-4 rel err); the gate matmul runs in exact
float32 on the same SBUF bytes via bitcast so the top-2 selection matches
the fp32 reference.
"""

import numpy as np

import concourse.mybir as mybir
import concourse.tile as tile
from concourse import bacc
from concourse.bass_utils import run_bass_kernel_spmd

N_CORES = 8
B, T_SEQ, C = 8, 2048, 512
H, E = 2048, 4
N_TOK = B * T_SEQ            # 16384
T_CORE = N_TOK // N_CORES    # 2048 tokens per core
P = 128
CC = C // P                  # 4 contraction chunks for C
HC = H // P                  # 16 chunks for H
TC = T_CORE // P             # 16 token chunks
TB = 256                     # token block for the h staging buffer
NB = T_CORE // TB            # 8 blocks
TBC = TB // P                # 2 token chunks per block

F32 = mybir.dt.float32
F32R = mybir.dt.float32r


def build_module(reps: int = 1):
    nc = bacc.Bacc("TRN2", target_bir_lowering=False)

    xt = nc.dram_tensor("xt", [C, T_CORE], F32R, kind="ExternalInput")
    # Exact-fp32 copy of the same data for the gate: a DMA into an
    # fp32r-declared tensor rounds to fp32r in flight, so the gate streams its
    # x chunks through a separate genuinely-fp32 path.
    xg = nc.dram_tensor("xg", [C, T_CORE], F32, kind="ExternalInput")
    gw = nc.dram_tensor("gw", [C, E], F32, kind="ExternalInput")
    w1 = nc.dram_tensor("w1", [E, C, H], F32R, kind="ExternalInput")
    w2 = nc.dram_tensor("w2", [E, H, C], F32R, kind="ExternalInput")
    out = nc.dram_tensor("out", [T_CORE, C], F32, kind="ExternalOutput")
    rout = nc.dram_tensor("rout", [T_CORE, E], F32, kind="ExternalOutput")

    with tile.TileContext(nc) as tc:
        with (
            tc.tile_pool(name="xt_p", bufs=1) as xt_p,
            tc.tile_pool(name="gw_p", bufs=1) as gw_p,
            tc.tile_pool(name="rt_p", bufs=1) as rt_p,
            tc.tile_pool(name="acc_p", bufs=1) as acc_p,
            tc.tile_pool(name="w1_p", bufs=1) as w1_p,
            tc.tile_pool(name="w2_p", bufs=1) as w2_p,
            tc.tile_pool(name="h_p", bufs=2) as h_p,
            tc.tile_pool(name="gsmall", bufs=4) as gsmall,
            tc.tile_pool(name="ps_g", bufs=2, space="PSUM") as ps_g,
            tc.tile_pool(name="ps_h", bufs=4, space="PSUM") as ps_h,
            tc.tile_pool(name="ps_y", bufs=2, space="PSUM") as ps_y,
        ):

            def body():
                xt_sb = xt_p.tile([P, CC, T_CORE], F32R)
                nc.sync.dma_start(
                    out=xt_sb, in_=xt.ap().rearrange("(cc p) t -> p cc t", p=P)
                )
                gw_sb = gw_p.tile([P, CC, E], F32)
                nc.sync.dma_start(
                    out=gw_sb, in_=gw.ap().rearrange("(cc p) e -> p cc e", p=P)
                )
                router = rt_p.tile([P, TC, E], F32)
                acc = acc_p.tile([P, TC, C], F32)

                # ---- Gate: exact-fp32 logits, top-2 select, normalized weights.
                xg_ap = xg.ap().rearrange("(cc p) t -> p cc t", p=P)
                for t in range(TC):
                    xg_t = gsmall.tile([P, CC, P], F32, tag="xg")
                    nc.sync.dma_start(out=xg_t, in_=xg_ap[:, :, t * P : (t + 1) * P])
                    psg = ps_g.tile([P, E], F32)
                    for cc in range(CC):
                        nc.tensor.matmul(
                            psg,
                            xg_t[:, cc, :],
                            gw_sb[:, cc, :],
                            start=(cc == 0),
                            stop=(cc == CC - 1),
                        )
                    lg = gsmall.tile([P, E], F32, tag="lg")
                    nc.vector.tensor_copy(out=lg, in_=psg)
                    m1 = gsmall.tile([P, 1], F32, tag="m1")
                    nc.vector.tensor_reduce(
                        out=m1, in_=lg, axis=mybir.AxisListType.X, op=mybir.AluOpType.max
                    )
                    # msk = lg - 3e38*(lg >= m1) — top-1 pushed to -inf, the rest
                    # kept bit-exact (multiplying by an is_lt mask would rank 0
                    # above negative logits).
                    ind = gsmall.tile([P, E], F32, tag="ind")
                    nc.vector.tensor_scalar(
                        out=ind, in0=lg, scalar1=m1, scalar2=None, op0=mybir.AluOpType.is_ge
                    )
                    msk = gsmall.tile([P, E], F32, tag="msk")
                    nc.vector.scalar_tensor_tensor(
                        out=msk,
                        in0=ind,
                        scalar=-3.0e38,
                        in1=lg,
                        op0=mybir.AluOpType.mult,
                        op1=mybir.AluOpType.add,
                    )
                    m2 = gsmall.tile([P, 1], F32, tag="m2")
                    nc.vector.tensor_reduce(
                        out=m2, in_=msk, axis=mybir.AxisListType.X, op=mybir.AluOpType.max
                    )
                    nm2 = gsmall.tile([P, 1], F32, tag="nm2")
                    nc.scalar.mul(out=nm2, in_=m2, mul=-1.0)
                    # z = exp(lg - m2); keep top-2 only; normalize.
                    z = gsmall.tile([P, E], F32, tag="z")
                    nc.scalar.activation(
                        out=z, in_=lg, func=mybir.ActivationFunctionType.Exp, bias=nm2
                    )
                    keep = gsmall.tile([P, E], F32, tag="keep")
                    nc.vector.scalar_tensor_tensor(
                        out=keep,
                        in0=lg,
                        scalar=m2,
                        in1=z,
                        op0=mybir.AluOpType.is_ge,
                        op1=mybir.AluOpType.mult,
                    )
                    s = gsmall.tile([P, 1], F32, tag="s")
                    nc.vector.tensor_reduce(
                        out=s, in_=keep, axis=mybir.AxisListType.X, op=mybir.AluOpType.add
                    )
                    rs = gsmall.tile([P, 1], F32, tag="rs")
                    nc.vector.reciprocal(out=rs, in_=s)
                    nc.vector.tensor_scalar_mul(router[:, t, :], keep, rs)

                # ---- Experts: hT = gelu(w1[e].T @ xT) ; y = hT.T @ w2[e] ; combine.
                for e in range(E):
                    w1_sb = w1_p.tile([P, CC, H], F32R)
                    nc.sync.dma_start(
                        out=w1_sb, in_=w1.ap()[e].rearrange("(cc p) h -> p cc h", p=P)
                    )
                    w2_sb = w2_p.tile([P, HC, C], F32R)
                    nc.sync.dma_start(
                        out=w2_sb, in_=w2.ap()[e].rearrange("(hc p) c -> p hc c", p=P)
                    )
                    for b in range(NB):
                        h_sb = h_p.tile([P, HC, TB], F32R)
                        for hc in range(HC):
                            psh = ps_h.tile([P, TB], F32)
                            for cc in range(CC):
                                nc.tensor.matmul(
                                    psh,
                                    w1_sb[:, cc, hc * P : (hc + 1) * P],
                                    xt_sb[:, cc, b * TB : (b + 1) * TB],
                                    start=(cc == 0),
                                    stop=(cc == CC - 1),
                                )
                            nc.scalar.activation(
                                out=h_sb[:, hc, :],
                                in_=psh,
                                func=mybir.ActivationFunctionType.Gelu,
                            )
                        for j in range(TBC):
                            t = b * TBC + j
                            psy = ps_y.tile([P, C], F32)
                            for hc in range(HC):
                                nc.tensor.matmul(
                                    psy,
                                    h_sb[:, hc, j * P : (j + 1) * P],
                                    w2_sb[:, hc, :],
                                    start=(hc == 0),
                                    stop=(hc == HC - 1),
                                )
                            if e == 0:
                                nc.vector.tensor_scalar_mul(
                                    acc[:, t, :], psy, router[:, t, e : e + 1]
                                )
                            else:
                                nc.vector.scalar_tensor_tensor(
                                    out=acc[:, t, :],
                                    in0=psy,
                                    scalar=router[:, t, e : e + 1],
                                    in1=acc[:, t, :],
                                    op0=mybir.AluOpType.mult,
                                    op1=mybir.AluOpType.add,
                                )

                nc.sync.dma_start(
                    out=out.ap().rearrange("(t p) c -> p t c", p=P), in_=acc
                )
                nc.sync.dma_start(
                    out=rout.ap().rearrange("(t p) e -> p t e", p=P), in_=router
                )

            if reps == 1:
                body()
            else:
                with tc.For_i(0, reps):
                    body()

    nc.finalize()
    return nc


_module_cache: dict[int, object] = {}


def _get_module(reps: int = 1):
    if reps not in _module_cache:
        _module_cache[reps] = build_module(reps)
    return _module_cache[reps]


def make_in_maps(x, gate_w, w1, w2):
    xf = np.ascontiguousarray(x.reshape(N_TOK, C), dtype=np.float32)
    gate_w = np.ascontiguousarray(gate_w, dtype=np.float32)
    w1 = np.ascontiguousarray(w1, dtype=np.float32)
    w2 = np.ascontiguousarray(w2, dtype=np.float32)
    in_maps = []
    for s in range(N_CORES):
        shard = xf[s * T_CORE : (s + 1) * T_CORE]
        in_maps.append(
            {
                "xt": np.ascontiguousarray(shard.T),
                "xg": np.ascontiguousarray(shard.T),
                "gw": gate_w,
                "w1": w1,
                "w2": w2,
            }
        )
    return in_maps


def kernel(x, gate_w, w1, w2):
    nc = _get_module(reps=1)
    in_maps = make_in_maps(x, gate_w, w1, w2)
    res = run_bass_kernel_spmd(nc, in_maps, core_ids=list(range(N_CORES)))
    out = np.concatenate([r["out"] for r in res.results], axis=0)
    router = np.concatenate([r["rout"] for r in res.results], axis=0)
    return out.reshape(B, T_SEQ, C), router
